# revision 21
# baseline (speedup 1.0000x reference)
"""Trainium2 Bass kernel for nn_BaselineAttention (B=2, N=2048, IN=512, D=1024, H=16, V=1).

Algorithm (restructured from the naive reference):
  scores_h = (h Wq_h)(h Wk_h)^T / sqrt(D) = h A_h h^T with A_h = Wq_h Wk_h^T/sqrt(D)
  (A_h precomputed on host: halves the projection FLOPs, 1 projection instead of 2)
  ctx[b,n,h] = softmax(scores_h) @ (h Wv_h)  (a scalar per head, V-dim is 1)
  out = ctx @ (Wo @ dec_w) + dec_b           (Wo@dec_w = M[16,1024] folded on host)

Sharding: core c -> batch b=c//4, q-shard qs=c%4 (rows qs*512..+512), ALL 16 heads.
No collectives: each core computes its output shard completely locally.
To keep the program SPMD-uniform, each core's xT/cT are rolled by -qs*512 along n
so its q-shard sits at local columns 0..512; the causal-free "mask col 0" becomes a
per-core [128,16] scale-mask input (exp(0*s)=1 reproduces the reference's
multiplicative mask + softmax semantics).

Per core pipeline (all bf16 matmuls except the f32r embedding):
  hT  = (emb_w.T @ xT + cT)           [128, 8dc, 2048]  bf16
  v   = hT.T @ Wv (all 16 heads)      -> lv[:,kt,(1,v_h)] stacked lhsT columns
  per head: tT = A_h^T-contraction    [128, 8eb, 512]    (q-shard only)
            sT tiles [128k,512q] = hTb.T @ tT  (psum), exp via ACT (scale-mask)
            [den;num] = [ones,v_h]^T @ exp(sT)  accumulated over 16 k-chunks (PE)
            ctxT[h,:] = num/den  (DVE)
  out = ctxT.T @ M + dec_b            [512, 1024]
Host reassembles the 8 shards into [2, 2048, 1024].

bench() runs a NEFF with a hardware For_i loop (LOOP_K iterations per launch) to
amortize the ~58ms axon dispatch overhead; reported HW exec time = wall/LOOP_K.
"""
import numpy as np

import concourse.bass as bass
import concourse.mybir as mybir
import concourse.tile as tile
from concourse import bacc
from concourse.bass_utils import run_bass_kernel_spmd

F32 = mybir.dt.float32
F32R = mybir.dt.float32r
BF16 = mybir.dt.bfloat16
AX = mybir.AxisListType
OP = mybir.AluOpType
ACTF = mybir.ActivationFunctionType

N_CORES = 8
B, N, IN, D, H, NCLS = 2, 2048, 512, 1024, 16, 1024
P = 128
DC = D // P          # 8 d-chunks (contraction for tT)
IC = IN // P         # 4 in-chunks (embedding contraction)
KT = N // P          # 16 k-tiles of 128
NQ = N // 4          # 512 local q columns
SCALE = 1.0 / np.sqrt(np.float32(D))
LOOP_K = 2000
UNROLL = 1


def build(loop_k: int = 1):
    nc = bacc.Bacc("TRN2", target_bir_lowering=False, debug=False, num_devices=N_CORES)

    xT = nc.dram_tensor("xT", [IN, N], F32R, kind="ExternalInput").ap()
    cT = nc.dram_tensor("cT", [D, N], BF16, kind="ExternalInput").ap()
    emb_w = nc.dram_tensor("emb_w", [IN, D], F32R, kind="ExternalInput").ap()
    amat_d = nc.dram_tensor("amat_d", [H, D, D], BF16, kind="ExternalInput").ap()
    wv = nc.dram_tensor("wv", [D, H], BF16, kind="ExternalInput").ap()
    mmat = nc.dram_tensor("mmat", [H, NCLS], F32R, kind="ExternalInput").ap()
    decb = nc.dram_tensor("decb", [1, NCLS], F32, kind="ExternalInput").ap()
    smask = nc.dram_tensor("smask", [P, KT], F32, kind="ExternalInput").ap()
    out = nc.dram_tensor("out", [NQ, NCLS], F32, kind="ExternalOutput").ap()

    from contextlib import ExitStack
    import contextlib

    with tile.TileContext(nc) as tc:
        with ExitStack() as es:
            cst = es.enter_context(tc.tile_pool(name="cst", bufs=1))
            xtp = es.enter_context(tc.tile_pool(name="xt", bufs=2))
            ap_ = es.enter_context(tc.tile_pool(name="ap", bufs=3))
            ttp = es.enter_context(tc.tile_pool(name="tt", bufs=2))
            ptp = es.enter_context(tc.tile_pool(name="pt", bufs=4))
            recp = es.enter_context(tc.tile_pool(name="rec", bufs=2))
            finp = es.enter_context(tc.tile_pool(name="fin", bufs=2))
            accp = es.enter_context(tc.tile_pool(name="acc", bufs=3, space="PSUM"))
            scp = es.enter_context(tc.tile_pool(name="sc", bufs=3, space="PSUM"))
            s2p = es.enter_context(tc.tile_pool(name="s2", bufs=2, space="PSUM"))

            # ---- constants loaded once (outside the loop)
            embw = cst.tile([P, IC, D], F32R, tag="embw")
            nc.sync.dma_start(embw[:], emb_w.rearrange("(ic p) d -> p ic d", p=P))
            wv_sb = cst.tile([P, DC, H], BF16, tag="wv")
            nc.sync.dma_start(wv_sb[:], wv.rearrange("(dc p) h -> p dc h", p=P))
            m_sb = cst.tile([H, NCLS], F32R, tag="m")
            nc.sync.dma_start(m_sb[:], mmat[:])
            dbb = cst.tile([P, NCLS], F32, tag="dbb")
            nc.sync.dma_start(dbb[:], decb[0].partition_broadcast(P))
            sm_sb = cst.tile([P, KT], F32, tag="sm")
            nc.sync.dma_start(sm_sb[:], smask[:])
            ctb = cst.tile([P, DC, N], BF16, tag="ctb")
            nc.sync.dma_start(ctb[:], cT.rearrange("(dc p) n -> p dc n", p=P))

            hTb = cst.tile([P, DC, N], BF16, tag="hTb")
            # lv[:, x, h]: x=2*kt -> 1.0 (ones), x=2*kt+1 -> v_h[kt chunk].
            # Padded to X=159 so a [128]-wide strided lhsT slice starting at
            # x=2*kt stays in bounds: rows 2..127 of the ps2 psum are garbage
            # from later kt weights, only rows 0:2 (den,num) are read.
            LVX = 2 * KT + 127
            lv = cst.tile([P, LVX, H], BF16, tag="lv")
            nc.vector.memset(lv[:], 0.0)
            nc.vector.memset(
                lv[:, 0 : 2 * KT, :].rearrange("p (k two) h -> p k two h", two=2)[
                    :, :, 0, :
                ],
                1.0,
            )
            ctxT = cst.tile([H, NQ], F32R, tag="ctxT")

            def emit_pass():
                # ---- embedding: hTb[dc, n] = sum_ic emb_w[ic,dc].T @ xT[ic,n] + cT
                for nch in range(4):
                    xt = xtp.tile([P, IC, 512], F32R)
                    nc.sync.dma_start(
                        xt[:],
                        xT[:, nch * 512 : (nch + 1) * 512].rearrange(
                            "(ic p) n -> p ic n", p=P
                        ),
                    )
                    for dc in range(DC):
                        ps = accp.tile([P, 512], F32, tag="acc")
                        for ic in range(IC):
                            nc.tensor.matmul(
                                ps[:], embw[:, ic, dc * P : (dc + 1) * P],
                                xt[:, ic, :], start=(ic == 0), stop=(ic == IC - 1),
                            )
                        nc.vector.tensor_tensor(
                            hTb[:, dc, nch * 512 : (nch + 1) * 512], ps[:],
                            ctb[:, dc, nch * 512 : (nch + 1) * 512], OP.add,
                        )

                # ---- V for all 16 heads -> lv[:, 2kt+1, h]=v_h (ones preset)
                for kt in range(KT):
                    pv = accp.tile([P, H], F32, tag="acc")
                    for dc in range(DC):
                        nc.tensor.matmul(
                            pv[:], hTb[:, dc, kt * P : (kt + 1) * P],
                            wv_sb[:, dc, :], start=(dc == 0), stop=(dc == DC - 1),
                        )
                    nc.scalar.copy(lv[:, 2 * kt + 1, :], pv[:])

                # ---- per head: tT, scores, exp, [den;num], ctx
                for hh in range(H):
                    am = ap_.tile([P, DC, D], BF16, tag="am")
                    nc.sync.dma_start(
                        am[:], amat_d[hh].rearrange("(dc p) e -> p dc e", p=P)
                    )
                    tt = ttp.tile([P, DC, NQ], BF16, tag="tt")
                    for eb in range(DC):
                        pt_ = accp.tile([P, NQ], F32, tag="acc")
                        for dc in range(DC):
                            nc.tensor.matmul(
                                pt_[:], am[:, dc, eb * P : (eb + 1) * P],
                                hTb[:, dc, 0:NQ], start=(dc == 0), stop=(dc == DC - 1),
                            )
                        nc.vector.tensor_copy(tt[:, eb, :], pt_[:])

                    ps2 = s2p.tile([P, NQ], F32, tag="s2")
                    for kt in range(KT):
                        ps = scp.tile([P, NQ], F32, tag="sc")
                        for eb in range(DC):
                            nc.tensor.matmul(
                                ps[:], hTb[:, eb, kt * P : (kt + 1) * P],
                                tt[:, eb, :], start=(eb == 0), stop=(eb == DC - 1),
                            )
                        pe_t = ptp.tile([P, NQ], BF16)
                        nc.scalar.activation(
                            pe_t[:], ps[:], ACTF.Exp,
                            bias=0.0, scale=sm_sb[:, kt : kt + 1],
                        )
                        nc.tensor.matmul(
                            ps2[:], lv[:, 2 * kt : 2 * kt + P, hh], pe_t[:],
                            start=(kt == 0), stop=(kt == KT - 1),
                        )
                    sb2 = recp.tile([2, NQ], F32, tag="sb2")
                    nc.scalar.copy(sb2[:], ps2[0:2, :])
                    dn = recp.tile([1, 2, NQ], F32, tag="dn")
                    nc.sync.dma_start(dn[:], sb2[:])
                    rec = recp.tile([1, NQ], F32, tag="rec")
                    nc.vector.reciprocal(rec[:], dn[:, 0, :])
                    crow = recp.tile([1, NQ], F32R, tag="crow")
                    nc.vector.tensor_tensor(crow[:], dn[:, 1, :], rec[:], OP.mult)
                    nc.sync.dma_start(ctxT[hh : hh + 1, :], crow[:])

                # ---- decode: out = ctxT.T @ M + dec_b
                for qt in range(4):
                    for cb in range(2):
                        pd = accp.tile([P, 512], F32, tag="acc")
                        nc.tensor.matmul(
                            pd[:], ctxT[:, qt * P : (qt + 1) * P],
                            m_sb[:, cb * 512 : (cb + 1) * 512], start=True, stop=True,
                        )
                        fin = finp.tile([P, 512], F32)
                        nc.vector.tensor_tensor(
                            fin[:], pd[:], dbb[:, cb * 512 : (cb + 1) * 512], OP.add
                        )
                        nc.sync.dma_start(
                            out[qt * P : (qt + 1) * P, cb * 512 : (cb + 1) * 512],
                            fin[:],
                        )

            if loop_k > 1:
                assert loop_k % UNROLL == 0
                with tc.For_i(0, loop_k // UNROLL, 1):
                    for _ in range(UNROLL):
                        emit_pass()
            else:
                emit_pass()
    nc.compile()
    return nc


_NC = {}


def _get_nc(loop_k=1):
    if loop_k not in _NC:
        _NC[loop_k] = build(loop_k)
    return _NC[loop_k]


def _pos_encoding():
    pos = np.arange(N, dtype=np.float32)[:, None]
    div = np.exp(
        np.arange(0, D, 2, dtype=np.float32) * np.float32(-np.log(10000.0) / D)
    ).astype(np.float32)
    pe = np.zeros((N, D), dtype=np.float32)
    pe[:, 0::2] = np.sin(pos * div)
    pe[:, 1::2] = np.cos(pos * div)
    return pe


def make_in_maps(X, emb_w, emb_b, Wq, Wk, Wv, Wo, dec_w, dec_b):
    import ml_dtypes

    X = np.asarray(X, dtype=np.float32)
    emb_w = np.ascontiguousarray(np.asarray(emb_w, dtype=np.float32))
    emb_b = np.asarray(emb_b, dtype=np.float32)
    Wq = np.asarray(Wq, dtype=np.float32)
    Wk = np.asarray(Wk, dtype=np.float32)
    Wv = np.asarray(Wv, dtype=np.float32)
    Wo = np.asarray(Wo, dtype=np.float32)
    dec_w = np.asarray(dec_w, dtype=np.float32)
    dec_b = np.asarray(dec_b, dtype=np.float32)

    pe = _pos_encoding()
    cT_base = (pe + emb_b[None, :]).T.astype(np.float32)          # [D, N]
    amat = np.ascontiguousarray(
        (np.matmul(Wq, np.transpose(Wk, (0, 2, 1))) * np.float32(SCALE)).astype(
            ml_dtypes.bfloat16
        )
    )                                                              # [H, D, D]
    wv_t = np.ascontiguousarray(Wv[:, :, 0].T.astype(ml_dtypes.bfloat16))  # [D, H]
    mmat = np.ascontiguousarray((Wo @ dec_w).astype(np.float32))  # [H, NCLS]
    decb = np.ascontiguousarray(dec_b[None, :].astype(np.float32))

    in_maps = []
    for c in range(N_CORES):
        b = c // 4
        qs = c % 4
        roll = -qs * 512
        xTr = np.ascontiguousarray(np.roll(X[b].T, roll, axis=1))
        cTr = np.ascontiguousarray(np.roll(cT_base, roll, axis=1).astype(ml_dtypes.bfloat16))
        sm = np.ones((P, KT), dtype=np.float32)
        j0 = ((4 - qs) % 4) * 512          # local column of global k=0
        sm[0, j0 // P] = 0.0
        in_maps.append({
            "xT": xTr,
            "cT": cTr,
            "emb_w": emb_w,
            "amat_d": amat,
            "wv": wv_t,
            "mmat": mmat,
            "decb": decb,
            "smask": sm,
        })
    return in_maps


def run(trace=False, loop_k=1, **inputs):
    nc = _get_nc(loop_k)
    in_maps = make_in_maps(**inputs)
    res = run_bass_kernel_spmd(
        nc, in_maps, core_ids=list(range(N_CORES)), trace=trace
    )
    full = np.empty((B, N, NCLS), dtype=np.float32)
    for c in range(N_CORES):
        full[c // 4, (c % 4) * 512 : (c % 4 + 1) * 512, :] = res.results[c]["out"]
    return full, res


def kernel(**inputs):
    full, _ = run(trace=False, **inputs)
    return full


def bench(iters=10, loop_k=LOOP_K, nc=None, **inputs):
    """Time on-device NEFF execution. The NEFF runs loop_k full forward passes
    per launch (hardware For_i loop) to amortize dispatch overhead; returned
    times are per-pass (wall / loop_k)."""
    import time

    import jax
    import concourse.mybir as _mybir
    from concourse import bass2jax as b2j
    from jax.sharding import Mesh, PartitionSpec, NamedSharding
    from jax.experimental.shard_map import shard_map

    if nc is None:
        nc = _get_nc(loop_k)
    in_maps = make_in_maps(**inputs)
    b2j.install_neuronx_cc_hook()

    in_names, out_names, out_avals, zero_outs = [], [], [], []
    for alloc in nc.m.functions[0].allocations:
        if not isinstance(alloc, _mybir.MemoryLocationSet):
            continue
        name = alloc.memorylocations[0].name
        if alloc.kind == "ExternalInput":
            if not nc.partition_id_tensor or name != nc.partition_id_tensor.name:
                in_names.append(name)
        elif alloc.kind == "ExternalOutput":
            shape = tuple(alloc.tensor_shape)
            dtype = _mybir.dt.np(alloc.dtype)
            out_names.append(name)
            out_avals.append(jax.core.ShapedArray(shape, dtype))
            zero_outs.append(np.zeros(shape, dtype))
    n_params = len(in_names)
    all_in = list(in_names) + list(out_names)
    if nc.partition_id_tensor:
        all_in.append(nc.partition_id_tensor.name)

    def _body(*args):
        operands = list(args)
        if nc.partition_id_tensor:
            operands.append(b2j.partition_id_tensor())
        return tuple(
            b2j._bass_exec_p.bind(
                *operands,
                out_avals=tuple(out_avals),
                in_names=tuple(all_in),
                out_names=tuple(out_names),
                lowering_input_output_aliases=(),
                sim_require_finite=True,
                sim_require_nnan=True,
                nc=nc,
            )
        )

    devices = jax.devices()[:N_CORES]
    mesh = Mesh(np.asarray(devices), ("core",))
    nin = n_params + len(out_names)
    sharded = jax.jit(
        shard_map(
            _body, mesh=mesh, in_specs=(PartitionSpec("core"),) * nin,
            out_specs=(PartitionSpec("core"),) * len(out_names), check_rep=False,
        ),
        keep_unused=True,
    )
    sh = NamedSharding(mesh, PartitionSpec("core"))
    dev_in = [
        jax.device_put(
            np.concatenate([np.asarray(in_maps[c][k]) for c in range(N_CORES)], 0), sh
        )
        for k in in_names
    ] + [
        jax.device_put(np.zeros((N_CORES * z.shape[0], *z.shape[1:]), z.dtype), sh)
        for z in zero_outs
    ]
    outs = sharded(*dev_in)
    jax.block_until_ready(outs)  # warmup/compile
    times = []
    for _ in range(iters):
        t0 = time.perf_counter()
        outs = sharded(*dev_in)
        jax.block_until_ready(outs)
        times.append((time.perf_counter() - t0) / loop_k)
    full = np.empty((B, N, NCLS), dtype=np.float32)
    o = np.asarray(outs[out_names.index("out")]).reshape(N_CORES, N // 4, NCLS)
    for c in range(N_CORES):
        full[c // 4, (c % 4) * 512 : (c % 4 + 1) * 512, :] = o[c]
    return full, times
